# revision 16
# baseline (speedup 1.0000x reference)
"""GraphConv (DGL norm='both' + relu) Trainium2 kernel, 8-core SPMD.

out = relu( D_dst^{-1/2} A D_src^{-1/2} X W + b )

Strategy (per sharding hint): nodes are partitioned across the 8 cores;
edges are partitioned by destination node so the segment-sum scatter is
device-local; x is replicated so source features are gathered directly
from HBM (the "all-gather" is done at input-distribution time); W/b are
replicated.

Two device variants:

"hstat" (default, 16-bit): per 128-node block,
  - indirect-DMA gather of the block's edge source rows H [128e, 512] fp16
  - DVE builds S[e, n] = (dstloc[e]==n) * edgew[e] where
    edgew = nsrc[src]*ndst[dst] is folded on the host (index-space only)
  - PE: aggT[f, n] += H_chunk.T @ S per 128-feat chunk (the segment sum,
    feature-major so NO transposes are needed before the GEMM)
  - DVE: aggT PSUM -> SBUF fp16
  - PE: out = aggT.T @ W (+ b via a K=1 ones-matmul only when b != 0),
    ACT relu, fp16 store (host upcasts).
  The GEMM of block i-1 is emitted between the scatters of block i so the
  PE never waits on the PSUM->SBUF copy.

"base" (fp32r): the original scatter->transpose->GEMM pipeline.

Host does only index-space preprocessing: degree counts (bincount),
balanced node->block assignment, edge bucketing/padding, per-edge norm
weights, and the final inverse permutation of the output rows.
"""

import heapq
import os
import sys

import numpy as np

sys.path.insert(0, "/opt/trn_rl_repo")

P = 128          # partitions / nodes per block
N_CORES = 8
F_IN = 512
F_OUT = 512
K_CH = F_IN // P  # 4 contraction chunks in the GEMM
GATHER_DTYPE = "fp16"   # "f32r" | "fp16" | "bf16"
VARIANT = "hstat"       # "hstat" | "base"
OUT_16 = True           # store outputs 16-bit, upcast on host
N_QUEUES = 2
GB = 1                  # blocks per batched indirect gather launch
                        # (384 descriptors/launch; larger overflows the
                        # SWDGE ring and lands rows wrong)

_PROGRAM_CACHE = {}


# ----------------------------------------------------------------------------
# host-side graph preprocessing (index-space only)
# ----------------------------------------------------------------------------

def _preprocess(src, dst, n_nodes):
    """Balanced node->block assignment + edge bucketing.

    Returns dict with per-core index arrays and the node permutation.
    """
    E = src.shape[0]
    bpc = int(np.ceil(n_nodes / (N_CORES * P)))      # blocks per core
    nblocks = N_CORES * bpc
    npad = nblocks * P

    deg_out = np.bincount(src, minlength=n_nodes).astype(np.int64)
    deg_in = np.bincount(dst, minlength=n_nodes).astype(np.int64)

    # Greedy balanced assignment of nodes to blocks (minimize max block
    # in-edge count so every block needs the same number of edge chunks).
    order = np.argsort(-deg_in, kind="stable")
    block_of = np.empty(n_nodes, np.int64)
    slot_of = np.empty(n_nodes, np.int64)
    counts = np.zeros(nblocks, np.int64)
    heap = [(0, b) for b in range(nblocks)]
    heapq.heapify(heap)
    deg_in_l = deg_in.tolist()
    for n in order.tolist():
        load, b = heapq.heappop(heap)
        block_of[n] = b
        slot_of[n] = counts[b]
        counts[b] += 1
        if counts[b] < P:
            heapq.heappush(heap, (load + deg_in_l[n], b))

    node_order = np.full(npad, -1, np.int64)
    node_order[block_of * P + slot_of] = np.arange(n_nodes)

    # Edge bucketing by destination block.
    eblk = block_of[dst]
    ec = np.bincount(eblk, minlength=nblocks)
    C = max(1, int(np.ceil(ec.max() / P)))           # chunks per block
    ek = np.lexsort((src, eblk))                      # group by block, then src
    eblk_s = eblk[ek]
    starts = np.concatenate(([0], np.cumsum(ec)))
    pos = np.arange(E) - starts[eblk_s]
    chunk = pos // P
    part = pos % P
    core = eblk_s // bpc
    blk_in_core = eblk_s % bpc
    col = blk_in_core * C + chunk

    # per-edge combined norm weight: nsrc[src] * ndst[dst]
    nsrc_n = 1.0 / np.sqrt(np.where(deg_out > 0, deg_out, 1))
    ndst_n = 1.0 / np.sqrt(np.where(deg_in > 0, deg_in, 1))
    ew = (nsrc_n[src] * ndst_n[dst]).astype(np.float32)

    ncols = bpc * C
    src_idx = np.zeros((N_CORES, P, ncols), np.int32)
    dstloc = np.full((N_CORES, P, ncols), -1.0, np.float32)
    degsrc = np.ones((N_CORES, P, ncols), np.float32)
    edgew = np.zeros((N_CORES, P, ncols), np.float32)
    src_idx[core, part, col] = src[ek].astype(np.int32)
    dstloc[core, part, col] = slot_of[dst[ek]].astype(np.float32)
    degsrc[core, part, col] = deg_out[src[ek]].astype(np.float32)
    edgew[core, part, col] = ew[ek]

    # per-node in-degree, laid out [core, slot(partition), block]
    deg_in_pad = np.ones(npad, np.float32)
    valid = node_order >= 0
    d = deg_in[node_order[valid]]
    deg_in_pad[valid] = np.where(d > 0, d, 1).astype(np.float32)
    degin = np.transpose(deg_in_pad.reshape(N_CORES, bpc, P), (0, 2, 1)).copy()

    try:
        return dict(_dg_layout(src, dst, n_nodes, bpc, C, npad, node_order,
                               src_idx, dstloc, degsrc, degin, edgew,
                               ek, ec, starts, slot_of, ew, nblocks),
                    dg_ok=True)
    except Exception:
        return dict(bpc=bpc, C=C, npad=npad, node_order=node_order,
                    src_idx=src_idx, dstloc=dstloc, degsrc=degsrc,
                    degin=degin, edgew=edgew, dg_ok=False)


def _dg_layout(src, dst, n_nodes, bpc, C, npad, node_order, src_idx, dstloc,
               degsrc, degin, edgew, ek, ec, starts, slot_of, ew, nblocks):
    # ---- dma_gather ("dg") layout: per block, 2 low chunks + 2 high chunks.
    # low gathers read x[idx] (idx = src, needs src < XSPLIT = int16 max+1);
    # high gathers read xhi = x[XHI0:] (idx = src - XHI0, needs src >= XHI0).
    # Edges with src in [XHI0, XSPLIT) can go on either side, which makes the
    # <=256-per-side split always satisfiable.
    XSPLIT, XHI0 = 32768, n_nodes - 32768  # xhi covers [XHI0, n_nodes)
    CD = 4
    SIDE = 2 * P  # 256 slots per side
    dst4 = np.full((N_CORES, P, bpc * CD), -1.0, np.float32)
    ew4 = np.zeros((N_CORES, P, bpc * CD), np.float32)
    idxlo = np.zeros((N_CORES, bpc, SIDE), np.int16)
    idxhi = np.zeros((N_CORES, bpc, SIDE), np.int16)
    src_s = src[ek]
    dst_slot_s = slot_of[dst[ek]]
    ew_s = ew[ek]
    starts_b = starts  # per global block
    for gb in range(nblocks):
        core_, blk = gb // bpc, gb % bpc
        e0, e1 = starts_b[gb], starts_b[gb] + ec[gb]
        s_b = src_s[e0:e1]
        d_b = dst_slot_s[e0:e1]
        w_b = ew_s[e0:e1]
        is_low = (s_b < XHI0) | ((s_b < XSPLIT) &
                                 (np.cumsum(s_b < XSPLIT) <= SIDE))
        lo = np.nonzero(is_low)[0]
        hi = np.nonzero(~is_low)[0]
        assert len(lo) <= SIDE and len(hi) <= SIDE, (len(lo), len(hi))
        assert (s_b[lo] < XSPLIT).all() and (s_b[hi] >= XHI0).all()
        for sel, secbase, idxarr, off in (
                (lo, 0, idxlo, 0), (hi, 2, idxhi, XHI0)):
            j = np.arange(len(sel))
            cols = blk * CD + secbase + j // P
            parts = j % P
            dst4[core_, parts, cols] = d_b[sel].astype(np.float32)
            ew4[core_, parts, cols] = w_b[sel]
            idxarr[core_, blk, :len(sel)] = (s_b[sel] - off).astype(np.int16)

    # group blocks (G per gather pair) and wrap idx lists into the int16
    # [128, n/16] layout (16-partition wrap, replicated 8x for the Q7 cores)
    G = 7
    ngroups = (bpc + G - 1) // G
    gsizes = [min(G, bpc - g * G) for g in range(ngroups)]
    def _wrap(idxarr):
        out = []
        for core_ in range(N_CORES):
            cols = []
            for g in range(ngroups):
                lst = idxarr[core_, g * G:g * G + gsizes[g]].reshape(-1)
                w16 = lst.reshape(-1, 16).T  # [16, n/16]
                cols.append(np.tile(w16, (8, 1)))
            out.append(np.concatenate(cols, axis=1))
        return np.stack(out)  # [N_CORES, 128, bpc*SIDE/16]
    idxlo_w = _wrap(idxlo)
    idxhi_w = _wrap(idxhi)

    return dict(
        bpc=bpc, C=C, npad=npad, node_order=node_order,
        src_idx=src_idx, dstloc=dstloc, degsrc=degsrc, degin=degin,
        edgew=edgew, dst4=dst4, ew4=ew4, idxlo=idxlo_w, idxhi=idxhi_w,
        G=G, gsizes=gsizes, xhi0=XHI0,
    )


# ----------------------------------------------------------------------------
# device program
# ----------------------------------------------------------------------------

def _indirect_gather_q(eng, out, in_, offset_ap, queue):
    """nc.gpsimd.indirect_dma_start (gather form), with a SWDGE queue choice.

    Replicates bass.BassGpSimd.indirect_dma_start's lowering but emits the
    InstDMACopy on qPoolDynamic{queue} so gathers can spread across multiple
    SWDGE contexts.
    """
    import concourse.mybir as mybir

    out_ap = eng.lower_ap_dma(out, for_indirect_dma=True)
    in_ap = eng.lower_ap_dma(in_, for_indirect_dma=True)
    assert len(in_ap) == 1 and len(out_ap) == 1
    offset_l = eng.lower_ap_dma(offset_ap)
    assert len(offset_l) == 1
    in_ap.append(offset_l[0])

    ap_shape = in_.shape
    coef = 1
    for i in range(1, len(ap_shape)):
        coef *= ap_shape[i]
    in_ap[0].dynamic_ap_info = mybir.DynamicAccessPatternInfo(
        c=0,
        actual_ap=out.ap,
        indirect_dim_max_index=ap_shape[0],
        offset_expr=[
            mybir.DynamicAccessPatternOffsetExpr(
                coef=coef,
                aff_expr=mybir.DynamicAccessPatternOffsetExprAffExpr(
                    kind="IndirectArgId", arg_id=1),
            )
        ],
    )
    return eng.add_instruction(
        mybir.InstDMACopy(
            name=eng.bass.get_next_instruction_name(),
            queue=f"qPoolDynamic{queue or ''}",
            mode="Copy",
            ins=in_ap,
            outs=out_ap,
            oob_is_err=True,
            cce_op=mybir.AluOpType.bypass,
        )
    )


def _build_program(n_nodes, bpc, C, repeat=1, ablate=(), n_queues=N_QUEUES,
                   gather_dtype=None, bufs_g=12, aggt_act=False,
                   variant=None, has_bias=False, out16=None, hw_loop=1,
                   gb=GB):
    import concourse.bass as bass
    import concourse.tile as tile
    from concourse import bacc, mybir
    from concourse.masks import make_identity

    if gather_dtype is None:
        gather_dtype = GATHER_DTYPE
    if variant is None:
        variant = VARIANT
    if out16 is None:
        out16 = OUT_16

    ablate = set(ablate)

    f32 = mybir.dt.float32
    f32r = mybir.dt.float32r
    i32 = mybir.dt.int32
    AF = mybir.ActivationFunctionType
    ALU = mybir.AluOpType
    gdt = {"f32r": f32r, "fp16": mybir.dt.float16,
           "bf16": mybir.dt.bfloat16}[gather_dtype]
    # 16-bit gather -> run the whole matmul pipeline (W, agg, transposes) in
    # the same 16-bit dtype: FWL fast weight loads + 1 cyc/row transposes.
    mdt = gdt if gather_dtype != "f32r" else f32r
    tdt = gdt if gather_dtype != "f32r" else f32  # transpose dtype
    odt = gdt if (out16 and gather_dtype != "f32r") else f32

    if variant in ("hstat", "dg"):
        assert gather_dtype != "f32r", "hstat needs 16-bit (moving dim 128)"

    CD = 4
    SIDE = 2 * P
    ncols = bpc * (CD if variant == "dg" else C)

    nc = bacc.Bacc("TRN2", target_bir_lowering=False, debug=False,
                   num_devices=N_CORES, num_swdge_queues=max(1, n_queues))

    x_d = nc.dram_tensor("x", [n_nodes, F_IN], gdt, kind="ExternalInput").ap()
    w_d = nc.dram_tensor("w", [F_IN, F_OUT], mdt, kind="ExternalInput").ap()
    if variant == "dg":
        i16 = mybir.dt.int16
        xhi_d = nc.dram_tensor("xhi", [32768, F_IN], gdt,
                               kind="ExternalInput").ap()
        nidxcols = bpc * SIDE // 16
        idxlo_d = nc.dram_tensor("idxlo", [P, nidxcols], i16,
                                 kind="ExternalInput").ap()
        idxhi_d = nc.dram_tensor("idxhi", [P, nidxcols], i16,
                                 kind="ExternalInput").ap()
    else:
        srcidx_d = nc.dram_tensor("src_idx", [P, ncols], i32,
                                  kind="ExternalInput").ap()
    dstloc_d = nc.dram_tensor("dstloc", [P, ncols], f32, kind="ExternalInput").ap()
    out_d = nc.dram_tensor("out", [bpc * P, F_OUT], odt, kind="ExternalOutput").ap()
    if variant == "base" or has_bias:
        b_d = nc.dram_tensor("b", [1, F_OUT], mdt, kind="ExternalInput").ap()
        ones_d = nc.dram_tensor("ones", [1, P], mdt, kind="ExternalInput").ap()
    if variant == "base":
        degsrc_d = nc.dram_tensor("degsrc", [P, ncols], f32, kind="ExternalInput").ap()
        degin_d = nc.dram_tensor("degin", [P, bpc], f32, kind="ExternalInput").ap()
    else:
        edgew_d = nc.dram_tensor("edgew", [P, ncols], f32, kind="ExternalInput").ap()

    with tile.TileContext(nc) as tc:
        with (
            tc.tile_pool(name="const", bufs=1) as cpool,
            tc.tile_pool(name="gpool",
                         bufs=2 if variant in ("dg", "hstat") else bufs_g
                         ) as gpool,
            tc.tile_pool(name="spool", bufs=12) as spool,
            tc.tile_pool(name="apool", bufs=4) as apool,
            tc.tile_pool(name="tpool", bufs=8) as tpool,
            tc.tile_pool(name="opool", bufs=3) as opool,
            tc.tile_pool(name="ps_agg", bufs=2, space="PSUM") as ps_agg,
            tc.tile_pool(name="ps_t", bufs=2, space="PSUM") as ps_t,
            tc.tile_pool(name="ps_out", bufs=2, space="PSUM") as ps_out,
        ):
            # ---- prologue: constants and index arrays
            w_t = cpool.tile([P, K_CH * F_OUT], mdt, tag="w")
            for k in range(K_CH):
                nc.sync.dma_start(
                    w_t[:, k * F_OUT:(k + 1) * F_OUT],
                    w_d[k * P:(k + 1) * P, :])
            if variant == "base" or has_bias:
                b_t = cpool.tile([1, F_OUT], mdt, tag="b")
                nc.sync.dma_start(b_t[:], b_d[:])
                ones_t = cpool.tile([1, P], mdt, tag="ones")
                nc.sync.dma_start(ones_t[:], ones_d[:])

            iota_i = cpool.tile([P, P], i32, tag="iota_i")
            nc.gpsimd.iota(iota_i[:], pattern=[[1, P]], base=0,
                           channel_multiplier=0)
            iota_f = cpool.tile([P, P], f32, tag="iota_f")
            nc.vector.tensor_copy(iota_f[:], iota_i[:])

            if variant == "dg":
                idxlo_t = cpool.tile([P, nidxcols], i16, tag="idxlo")
                nc.sync.dma_start(idxlo_t[:], idxlo_d[:])
                idxhi_t = cpool.tile([P, nidxcols], i16, tag="idxhi")
                nc.sync.dma_start(idxhi_t[:], idxhi_d[:])
            else:
                srcidx_t = cpool.tile([P, ncols], i32, tag="srcidx")
                nc.sync.dma_start(srcidx_t[:], srcidx_d[:])
            if variant == "hstat":
                # per-block index tiles: batched gathers need a full-tile
                # (zero-offset) multi-column offset AP
                srcb_tiles = []
                for i in range(bpc):
                    t = cpool.tile([P, C], i32, tag=f"srcb{i}")
                    nc.sync.dma_start(t[:], srcidx_d[:, i * C:(i + 1) * C])
                    srcb_tiles.append(t)
            dstloc_t = cpool.tile([P, ncols], f32, tag="dstloc")
            nc.sync.dma_start(dstloc_t[:], dstloc_d[:])

            if variant == "base":
                ident = cpool.tile([P, P], tdt, tag="ident")
                make_identity(nc, ident[:])
                degsrc_t = cpool.tile([P, ncols], f32, tag="degsrc")
                nc.sync.dma_start(degsrc_t[:], degsrc_d[:])
                degin_t = cpool.tile([P, bpc], f32, tag="degin")
                nc.sync.dma_start(degin_t[:], degin_d[:])
                # norms: n = sqrt(1/deg)
                wcol_t = cpool.tile([P, ncols], f32, tag="nsrc")
                nc.vector.reciprocal(wcol_t[:], degsrc_t[:])
                nc.scalar.activation(wcol_t[:], wcol_t[:], AF.Sqrt)
                ndst_t = cpool.tile([P, bpc], f32, tag="ndst")
                nc.vector.reciprocal(ndst_t[:], degin_t[:])
                nc.scalar.activation(ndst_t[:], ndst_t[:], AF.Sqrt)
            else:
                wcol_t = cpool.tile([P, ncols], f32, tag="edgew")
                nc.sync.dma_start(wcol_t[:], edgew_d[:])

            def gather(g, col):
                if "gather" in ablate:
                    # keep g written so Tile's read-tracking is satisfied
                    for k in range(K_CH):
                        nc.vector.tensor_scalar(
                            out=g[:, k * P:(k + 1) * P], in0=iota_f[:],
                            scalar1=0.0, scalar2=None, op0=ALU.mult)
                    return
                if n_queues <= 1:
                    nc.gpsimd.indirect_dma_start(
                        out=g[:], out_offset=None, in_=x_d[:],
                        in_offset=bass.IndirectOffsetOnAxis(
                            ap=srcidx_t[:, col:col + 1], axis=0),
                    )
                else:
                    _indirect_gather_q(
                        nc.gpsimd, g[:], x_d[:],
                        srcidx_t[:, col:col + 1], col % n_queues)

            def sbuild(sw, col):
                if "sbuild" in ablate:
                    return
                nc.vector.tensor_scalar(
                    out=sw[:], in0=iota_f[:],
                    scalar1=dstloc_t[:, col:col + 1],
                    scalar2=wcol_t[:, col:col + 1],
                    op0=ALU.is_equal, op1=ALU.mult)

            if variant in ("hstat", "dg"):
                def emit_gemm(state):
                    i, aggT_sb = state
                    p_out = ps_out.tile([P, F_OUT], mybir.dt.float32,
                                        tag="out")
                    if "gemm" not in ablate:
                        if has_bias:
                            nc.tensor.matmul(p_out[:], lhsT=ones_t[:1, :],
                                             rhs=b_t[:1, :], start=True,
                                             stop=False)
                        for k in range(K_CH):
                            nc.tensor.matmul(
                                p_out[:], lhsT=aggT_sb[:, k * P:(k + 1) * P],
                                rhs=w_t[:, k * F_OUT:(k + 1) * F_OUT],
                                start=(k == 0 and not has_bias),
                                stop=(k == K_CH - 1))
                    out_sb = opool.tile([P, F_OUT], odt, tag="out_sb")
                    if "gemm" not in ablate and "store" not in ablate:
                        nc.scalar.activation(out_sb[:], p_out[:], AF.Relu)
                        nc.sync.dma_start(out_d[i * P:(i + 1) * P, :],
                                          out_sb[:])

                def emit_block(i, state):
                    p_aggT = ps_agg.tile([P, F_IN], mybir.dt.float32,
                                         tag="aggT")
                    # one batched indirect gather for all C chunks of the
                    # block (amortizes the ~1us SWDGE launch overhead);
                    # plain queue + <=2 in flight, which probing validated
                    gbig = gpool.tile([P, C * F_IN], gdt, tag="g")
                    if "gather" in ablate:
                        for j in range(C * K_CH):
                            nc.vector.tensor_scalar(
                                out=gbig[:, j * P:(j + 1) * P],
                                in0=iota_f[:], scalar1=0.0,
                                scalar2=None, op0=ALU.mult)
                    else:
                        nc.gpsimd.indirect_dma_start(
                            out=gbig[:], out_offset=None, in_=x_d[:],
                            in_offset=bass.IndirectOffsetOnAxis(
                                ap=srcb_tiles[i][:], axis=0))
                    sws = []
                    for c in range(C):
                        col = i * C + c
                        sw = spool.tile([P, P], gdt, tag="sw")
                        sbuild(sw, col)
                        sws.append(sw)
                    if "scatmm" not in ablate:
                        # k-outer: each PSUM region's start..stop accumulation
                        # chain is contiguous (interleaved chains are broken
                        # in HW)
                        for k in range(K_CH):
                            for c in range(C):
                                nc.tensor.matmul(
                                    p_aggT[:, k * P:(k + 1) * P],
                                    lhsT=gbig[:, c * F_IN + k * P:
                                              c * F_IN + (k + 1) * P],
                                    rhs=sws[c][:],
                                    start=(c == 0), stop=(c == C - 1))
                    aggT_sb = apool.tile([P, F_IN], mdt, tag="aggT_sb")
                    if "scatmm" not in ablate and "copy" not in ablate:
                        nc.vector.tensor_copy(aggT_sb[:], p_aggT[:])
                    if state is not None:
                        emit_gemm(state)
                    return (i, aggT_sb) if "copy" not in ablate and \
                        "scatmm" not in ablate else None

                def emit_block_dg(i, b, gs, gt, state):
                    p_aggT = ps_agg.tile([P, F_IN], mybir.dt.float32,
                                         tag="aggT")
                    sws = []
                    for c in range(CD):
                        sw = spool.tile([P, P], gdt, tag="sw")
                        sbuild(sw, i * CD + c)
                        sws.append(sw)
                    if "scatmm" not in ablate:
                        for k in range(K_CH):
                            for c in range(CD):
                                ci = b * 2 + c if c < 2 else \
                                    gs * 2 + b * 2 + (c - 2)
                                nc.tensor.matmul(
                                    p_aggT[:, k * P:(k + 1) * P],
                                    lhsT=gt[:, ci, k * P:(k + 1) * P],
                                    rhs=sws[c][:],
                                    start=(c == 0), stop=(c == CD - 1))
                    aggT_sb = apool.tile([P, F_IN], mdt, tag="aggT_sb")
                    if "scatmm" not in ablate and "copy" not in ablate:
                        nc.vector.tensor_copy(aggT_sb[:], p_aggT[:])
                    if state is not None:
                        emit_gemm(state)
                    return (i, aggT_sb) if "copy" not in ablate and \
                        "scatmm" not in ablate else None

                GG = 7
                gsizes = [min(GG, bpc - g0 * GG)
                          for g0 in range((bpc + GG - 1) // GG)]

                def emit_group(g0, state):
                    gs = gsizes[g0]
                    gt = gpool.tile([P, gs * CD, F_IN], gdt, tag="g")
                    nci = gs * SIDE
                    co = sum(gsizes[:g0]) * SIDE // 16
                    if "gather" not in ablate:
                        nc.gpsimd.dma_gather(
                            gt[:, 0:gs * 2, :], x_d[:],
                            idxlo_t[:, co:co + nci // 16], nci, nci, F_IN)
                        nc.gpsimd.dma_gather(
                            gt[:, gs * 2:gs * 4, :], xhi_d[:],
                            idxhi_t[:, co:co + nci // 16], nci, nci, F_IN)
                    for b in range(gs):
                        state = emit_block_dg(g0 * GG + b, b, gs, gt, state)
                    return state

                def emit_pass():
                    state = None
                    if variant == "dg":
                        for g0 in range(len(gsizes)):
                            state = emit_group(g0, state)
                    else:
                        for i in range(bpc):
                            state = emit_block(i, state)
                    if state is not None:
                        emit_gemm(state)

                if hw_loop > 1:
                    with tc.For_i(0, hw_loop):
                        emit_pass()
                else:
                    for _ in range(repeat):
                        emit_pass()

            else:  # ---- base variant: scatter -> transpose -> GEMM
                def emit_block_base(i):
                    p_agg = ps_agg.tile([P, F_IN], mybir.dt.float32,
                                        tag="agg")
                    for c in range(C):
                        col = i * C + c
                        g = gpool.tile([P, F_IN], gdt, tag="g")
                        gather(g, col)
                        sw = spool.tile([P, P], gdt, tag="sw")
                        sbuild(sw, col)
                        if "scatmm" not in ablate:
                            nc.tensor.matmul(
                                p_agg[:], lhsT=sw[:], rhs=g[:],
                                start=(c == 0), stop=(c == C - 1))

                    # agg * ndst -> SBUF
                    agg_sb = apool.tile([P, F_IN], tdt, tag="agg_sb")
                    if "aggcopy" not in ablate and "scatmm" not in ablate:
                        nc.scalar.activation(agg_sb[:], p_agg[:], AF.Copy,
                                             scale=ndst_t[:, i:i + 1])

                    # transpose agg (feats onto partitions)
                    p_tr = ps_t.tile([P, F_IN], tdt, tag="tr")
                    aggT = tpool.tile([P, K_CH * P], mdt, tag="aggT")
                    if "transpose" not in ablate:
                        for k in range(K_CH):
                            nc.tensor.transpose(
                                p_tr[:, k * P:(k + 1) * P],
                                in_=agg_sb[:, k * P:(k + 1) * P],
                                identity=ident[:])
                        for k in range(K_CH):
                            if aggt_act:
                                nc.scalar.activation(
                                    aggT[:, k * P:(k + 1) * P],
                                    p_tr[:, k * P:(k + 1) * P], AF.Copy)
                            else:
                                nc.vector.tensor_copy(
                                    aggT[:, k * P:(k + 1) * P],
                                    p_tr[:, k * P:(k + 1) * P])

                    # GEMM + bias
                    p_out = ps_out.tile([P, F_OUT], mybir.dt.float32,
                                        tag="out")
                    if "gemm" not in ablate:
                        nc.tensor.matmul(p_out[:], lhsT=ones_t[:1, :],
                                         rhs=b_t[:1, :], start=True,
                                         stop=False)
                        for k in range(K_CH):
                            nc.tensor.matmul(
                                p_out[:], lhsT=aggT[:, k * P:(k + 1) * P],
                                rhs=w_t[:, k * F_OUT:(k + 1) * F_OUT],
                                start=False, stop=(k == K_CH - 1))

                    out_sb = opool.tile([P, F_OUT], odt, tag="out_sb")
                    if "gemm" not in ablate:
                        nc.scalar.activation(out_sb[:], p_out[:], AF.Relu)
                        nc.sync.dma_start(out_d[i * P:(i + 1) * P, :],
                                          out_sb[:])

                def emit_pass_base():
                    for i in range(bpc):
                        emit_block_base(i)

                if hw_loop > 1:
                    with tc.For_i(0, hw_loop):
                        emit_pass_base()
                else:
                    for _ in range(repeat):
                        emit_pass_base()

    nc.compile()
    return nc


# ----------------------------------------------------------------------------
# numpy emulation of the device program (for logic validation)
# ----------------------------------------------------------------------------

def _emulate(x, W, b, pre):
    bpc, C = pre["bpc"], pre["C"]
    outs = []
    iota = np.arange(P, dtype=np.float32)
    for core in range(N_CORES):
        src_idx = pre["src_idx"][core]
        dstloc = pre["dstloc"][core]
        edgew = pre["edgew"][core]
        out_core = np.empty((bpc * P, F_OUT), np.float32)
        for i in range(bpc):
            agg = np.zeros((P, F_IN), np.float32)
            for c in range(C):
                col = i * C + c
                g = x[src_idx[:, col]]
                sw = (iota[None, :] == dstloc[:, col:col + 1]) * \
                    edgew[:, col:col + 1]
                agg += sw.T.astype(np.float32) @ g
            out_core[i * P:(i + 1) * P] = np.maximum(agg @ W + b, 0.0)
        outs.append(out_core)
    return outs


# ----------------------------------------------------------------------------
# entry point
# ----------------------------------------------------------------------------

def _make_in_maps(x, W, b, pre, gather_dtype=None, variant=None,
                  has_bias=False, **_unused):
    if gather_dtype is None:
        gather_dtype = GATHER_DTYPE
    if variant is None:
        variant = VARIANT
    np_gdt = {"f32r": np.float32, "fp16": np.float16,
              "bf16": None}[gather_dtype]
    if np_gdt is None:
        import ml_dtypes
        np_gdt = ml_dtypes.bfloat16
    np_mdt = np.float32 if gather_dtype == "f32r" else np_gdt
    x = np.ascontiguousarray(x.astype(np_gdt))
    W = np.ascontiguousarray(W.astype(np_mdt))
    in_maps = []
    xhi = None
    if variant == "dg":
        xhi = np.ascontiguousarray(x[pre["xhi0"]:])
    for core in range(N_CORES):
        m = {"x": x, "w": W}
        if variant == "dg":
            m["xhi"] = xhi
            m["idxlo"] = np.ascontiguousarray(pre["idxlo"][core])
            m["idxhi"] = np.ascontiguousarray(pre["idxhi"][core])
            m["dstloc"] = np.ascontiguousarray(pre["dst4"][core])
            m["edgew"] = np.ascontiguousarray(pre["ew4"][core])
        else:
            m["src_idx"] = np.ascontiguousarray(pre["src_idx"][core])
            m["dstloc"] = np.ascontiguousarray(pre["dstloc"][core])
        if variant == "base" or has_bias:
            m["b"] = np.ascontiguousarray(
                b.reshape(1, F_OUT).astype(np_mdt))
            m["ones"] = np.ones((1, P), np_mdt)
        if variant == "base":
            m["degsrc"] = np.ascontiguousarray(pre["degsrc"][core])
            m["degin"] = np.ascontiguousarray(pre["degin"][core])
        elif variant != "dg":
            m["edgew"] = np.ascontiguousarray(pre["edgew"][core])
        in_maps.append(m)
    return in_maps


def _assemble(outs, pre, n_nodes):
    full = np.concatenate(outs, axis=0)           # [npad, F_OUT]
    node_order = pre["node_order"]
    valid = node_order >= 0
    result = np.empty((n_nodes, F_OUT), np.float32)
    result[node_order[valid]] = full[valid]
    return result


def kernel(x, src, dst, W, b):
    x = np.asarray(x)
    src = np.asarray(src).astype(np.int64)
    dst = np.asarray(dst).astype(np.int64)
    W = np.asarray(W)
    b = np.asarray(b)
    n_nodes = x.shape[0]

    pre = _preprocess(src, dst, n_nodes)

    if os.environ.get("GNN_KERNEL_EMULATE"):
        outs = _emulate(x.astype(np.float32), W.astype(np.float32),
                        b.astype(np.float32), pre)
        return _assemble(outs, pre, n_nodes)

    from concourse import bass_utils

    gather_dtype = os.environ.get("GNN_GATHER_DTYPE", GATHER_DTYPE)
    variant = os.environ.get("GNN_VARIANT", VARIANT)
    has_bias = bool(np.any(b != 0))
    key = (n_nodes, pre["bpc"], pre["C"], gather_dtype, variant, has_bias)
    if key not in _PROGRAM_CACHE:
        _PROGRAM_CACHE[key] = _build_program(
            n_nodes, pre["bpc"], pre["C"], gather_dtype=gather_dtype,
            variant=variant, has_bias=has_bias)
    nc = _PROGRAM_CACHE[key]

    in_maps = _make_in_maps(x, W, b, pre, gather_dtype=gather_dtype,
                            variant=variant, has_bias=has_bias)
    res = bass_utils.run_bass_kernel_spmd(
        nc, in_maps, core_ids=list(range(N_CORES)))
    outs = [res.results[c]["out"].astype(np.float32)
            for c in range(N_CORES)]
    return _assemble(outs, pre, n_nodes)

